# revision 9
# baseline (speedup 1.0000x reference)
# Cross-attention (single head) kernel for Trainium2, SPMD over 8 NeuronCores.
#
# Problem: nn_MultiHeadedAttention — B=16, Lq=1024, Lk=2048, D=768, fp32.
#   q = guide @ Wq.T + bq ; k = query @ Wk.T + bk ; v = query @ Wv.T + bv
#   out = softmax(q k^T / sqrt(D)) v ;  result = out @ Wo.T + bo
#
# Sharding: data-parallel over batch (2 batches per core), no collectives.
#
# Algebraic folding (host-side, exact):
#   - bk adds a per-query constant to all logits -> softmax-invariant, drop.
#   - A := Wq^T Wk / sqrt(D):  S = guide A query^T + c,  c = query (Wk^T bq)/sqrt(D)
#     (c is per-key, host-precomputed, applied as the exp() activation bias).
#     This eliminates both the q and k projections (one folded projection).
#   - C := Wo Wv:  result = (P_hat query) C^T + (bo + Wo bv)
#     This eliminates the v projection entirely; PV consumes raw query
#     embeds. Normalization by the softmax denominator is deferred to after
#     the C projection (linearity), so the reciprocal never blocks the PE.
#
# Layout strategy (zero on-device transposes): host uploads query embeds in
# both layouts (xT=[D,Lk] for S^T, x_nat=[Lk,D] for PV) and guide as
# gT=[D,Lq]; kernel computes S^T=[ik,iq] tiles so softmax denominators come
# from an all-ones [128,128] stationary matmul that directly produces the
# denominator broadcast across all partitions.
#
# All matmuls are bf16 operands with fp32 PSUM accumulation (measured
# end-to-end rel err vs fp32 reference ~3e-3).

import numpy as np
import ml_dtypes
from contextlib import ExitStack

import concourse.bass as bass
import concourse.tile as tile
from concourse import bacc, mybir
from concourse.bass_utils import run_bass_kernel_spmd

BF16 = mybir.dt.bfloat16
F32 = mybir.dt.float32

B, LQ, LK, D = 16, 1024, 2048, 768
NCORES = 8
BPC = B // NCORES          # batches per core = 2
NT = D // 128              # 6 feature tiles of 128
QC = 512                   # query chunk (free dim of S^T / PSUM bank width)
NQC = LQ // QC             # 2 query chunks
NIK = LK // 128            # 16 key tiles of 128
ACT_F = mybir.ActivationFunctionType


def build_nc():
    nc = bacc.Bacc("TRN2", target_bir_lowering=False, debug=False,
                   num_devices=NCORES)

    xT = nc.dram_tensor("xT", [BPC, D, LK], BF16, kind="ExternalInput").ap()
    xN = nc.dram_tensor("xN", [BPC, LK, D], BF16, kind="ExternalInput").ap()
    gT = nc.dram_tensor("gT", [BPC, D, LQ], BF16, kind="ExternalInput").ap()
    A = nc.dram_tensor("A", [D, D], BF16, kind="ExternalInput").ap()
    Ct = nc.dram_tensor("Ct", [D, D], BF16, kind="ExternalInput").ap()
    cb = nc.dram_tensor("cb", [BPC, LK], F32, kind="ExternalInput").ap()
    bof = nc.dram_tensor("bof", [D, 1], F32, kind="ExternalInput").ap()
    outT = nc.dram_tensor("outT", [BPC, D, LQ], F32, kind="ExternalOutput").ap()

    with tile.TileContext(nc) as tc, ExitStack() as ctx:
        _body(ctx, tc, outT, xT, xN, gT, A, Ct, cb, bof)
    nc.compile()
    return nc


def _body(ctx, tc, outT, xT, xN, gT, A, Ct, cb, bof):
    nc = tc.nc

    wts = ctx.enter_context(tc.tile_pool(name="wts", bufs=1))
    consts = ctx.enter_context(tc.tile_pool(name="consts", bufs=1))
    io = ctx.enter_context(tc.tile_pool(name="io", bufs=2))
    kqv = ctx.enter_context(tc.tile_pool(name="kqv", bufs=1))
    pt_pool = ctx.enter_context(tc.tile_pool(name="pt", bufs=17))
    ot_pool = ctx.enter_context(tc.tile_pool(name="ot", bufs=2))
    res_pool = ctx.enter_context(tc.tile_pool(name="res", bufs=4))
    rb_pool = ctx.enter_context(tc.tile_pool(name="rb", bufs=2))
    psum = ctx.enter_context(tc.tile_pool(name="psum", bufs=1, space="PSUM"))

    # --- weights/constants tiles (DMAs emitted in first-use order below) ---
    A_sb = wts.tile([128, NT, D], BF16, tag="A", name="A_sb")
    Ct_sb = wts.tile([128, NT, D], BF16, tag="Ct", name="Ct_sb")
    bof_sb = consts.tile([128, NT], F32, tag="bof", name="bof_sb")
    ones_sb = consts.tile([128, 128], BF16, tag="ones", name="ones_sb")
    nc.vector.memset(ones_sb, 1.0)

    for b in range(BPC):
        # --- load activations, ordered by first use so the PE can start as
        # soon as A + gT land (qp projection), then xT (S^T), then xN (PV),
        # then Ct (final projection) ---
        xT_sb = io.tile([128, NT, LK], BF16, tag="xT", name="xT_sb")
        xN_sb = io.tile([128, NIK, D], BF16, tag="xN", name="xN_sb")
        gT_sb = io.tile([128, NT, LQ], BF16, tag="gT", name="gT_sb")
        cb_sb = io.tile([128, NIK], F32, tag="cb", name="cb_sb")
        if b == 0:
            for td in range(NT):
                r = slice(td * 128, (td + 1) * 128)
                nc.sync.dma_start(out=A_sb[:, td, :], in_=A[r, :])
        for qc in range(NQC):
            cc = slice(qc * QC, (qc + 1) * QC)
            for td in range(NT):
                r = slice(td * 128, (td + 1) * 128)
                nc.sync.dma_start(out=gT_sb[:, td, cc], in_=gT[b, r, cc])
        for td in range(NT):
            r = slice(td * 128, (td + 1) * 128)
            nc.sync.dma_start(out=xT_sb[:, td, :], in_=xT[b, r, :])
        nc.sync.dma_start(out=cb_sb, in_=cb[b].rearrange("(t p) -> p t", p=128))
        for ik in range(NIK):
            r = slice(ik * 128, (ik + 1) * 128)
            nc.sync.dma_start(out=xN_sb[:, ik, :], in_=xN[b, r, :])
        if b == 0:
            for td in range(NT):
                r = slice(td * 128, (td + 1) * 128)
                nc.sync.dma_start(out=Ct_sb[:, td, :], in_=Ct[r, :])
            nc.sync.dma_start(out=bof_sb,
                             in_=bof.rearrange("(t p) o -> p (t o)", p=128))

        # --- folded q projection: qp^T[d', iq] = sum_d A[d, d'] gT[d, iq] ---
        qp_sb = kqv.tile([128, NT, LQ], BF16, tag="qp", name="qp_sb")
        for te in range(NT):
            ec = slice(te * 128, (te + 1) * 128)
            for qc in range(NQC):
                cc = slice(qc * QC, (qc + 1) * QC)
                ps = psum.tile([128, QC], F32, tag="acc", bufs=3, name="ps_q")
                for td in range(NT):
                    nc.tensor.matmul(ps, A_sb[:, td, ec], gT_sb[:, td, cc],
                                     start=(td == 0), stop=(td == NT - 1))
                nc.vector.tensor_copy(qp_sb[:, te, cc], ps)

        # --- attention, per query chunk of 512 ---
        for qc in range(NQC):
            cc = slice(qc * QC, (qc + 1) * QC)

            # denominator, broadcast over all 128 partitions by the all-ones
            # stationary operand: dn[m, iq] = sum_ik P^T[ik, iq] for every m
            dn = psum.tile([128, QC], F32, tag="dn", bufs=1, name="dn")
            ps_o = [psum.tile([128, QC], F32, tag="acc", bufs=3,
                              name=f"ps_o{t}") for t in range(3)]
            pts = []
            for ik in range(NIK):
                kc = slice(ik * 128, (ik + 1) * 128)
                # S^T[ik, iq] = sum_d' xT[d', ik] * qp^T[d', iq]
                ps_s = psum.tile([128, QC], F32, tag="s", bufs=3, name="ps_s")
                for te in range(NT):
                    nc.tensor.matmul(ps_s, xT_sb[:, te, kc], qp_sb[:, te, cc],
                                     start=(te == 0), stop=(te == NT - 1))
                # P^T = exp(S^T + c[ik])   (c: folded bq term, per-key)
                pt = pt_pool.tile([128, QC], BF16, tag="pt", name="pt")
                nc.scalar.activation(pt, ps_s, ACT_F.Exp,
                                     bias=cb_sb[:, ik:ik + 1])
                pts.append(pt)
                nc.tensor.matmul(dn, ones_sb, pt,
                                 start=(ik == 0), stop=(ik == NIK - 1))
                # PV wave 1: out^T[d, iq] += xN[ik, d] * P^T[ik, iq], d-tiles 0-2
                for t in range(3):
                    dc = slice(t * 128, (t + 1) * 128)
                    nc.tensor.matmul(ps_o[t], xN_sb[:, ik, dc], pt,
                                     start=(ik == 0), stop=(ik == NIK - 1))

            # wave-1 copies first: they release the acc PSUM slots wave 2
            # needs; the reciprocal (slow on DVE) runs after, overlapped
            # with wave-2 matmuls — it is only needed at the final muls.
            oT = ot_pool.tile([128, NT, QC], BF16, tag="oT", name="oT")
            for t in range(3):
                nc.vector.tensor_copy(oT[:, t, :], ps_o[t])

            # PV wave 2: d-tiles 3-5
            ps_o2 = [psum.tile([128, QC], F32, tag="acc", bufs=3,
                               name=f"ps_o2{t}") for t in range(3)]
            for ik in range(NIK):
                for t in range(3):
                    dc = slice((t + 3) * 128, (t + 4) * 128)
                    nc.tensor.matmul(ps_o2[t], xN_sb[:, ik, dc], pts[ik],
                                     start=(ik == 0), stop=(ik == NIK - 1))
            for t in range(3):
                nc.vector.tensor_copy(oT[:, t + 3, :], ps_o2[t])
            # reciprocal after the copies: DVE order matters — the copies
            # release PSUM slots the PE is waiting on; rb is only needed at
            # the final muls, well after the C matmuls start.
            rb = rb_pool.tile([128, QC], F32, tag="rb", name="rb")
            nc.vector.reciprocal(rb, dn)

            # result^T[f, iq] = (sum_d Ct[d, f] oT[d, iq]) / denom + bo'
            for tf in range(NT):
                fc = slice(tf * 128, (tf + 1) * 128)
                ps_r = psum.tile([128, QC], F32, tag="acc", bufs=3,
                                 name="ps_r")
                for te in range(NT):
                    nc.tensor.matmul(ps_r, Ct_sb[:, te, fc], oT[:, te, :],
                                     start=(te == 0), stop=(te == NT - 1))
                res = res_pool.tile([128, QC], F32, tag="res", name="res")
                nc.vector.tensor_mul(res, ps_r, rb)
                nc.scalar.activation(res, res, ACT_F.Identity,
                                     bias=bof_sb[:, tf:tf + 1])
                nc.sync.dma_start(out=outT[b, fc, cc], in_=res)


def _prep_in_maps(inputs):
    f32 = np.float32
    bf16 = ml_dtypes.bfloat16
    qe = np.asarray(inputs["query_embeds"], f32)    # [B, Lk, D]
    ge = np.asarray(inputs["guide_embeds"], f32)    # [B, Lq, D]
    Wq = np.asarray(inputs["Wq"], f32)
    Wk = np.asarray(inputs["Wk"], f32)
    Wv = np.asarray(inputs["Wv"], f32)
    Wo = np.asarray(inputs["Wo"], f32)
    bq = np.asarray(inputs["bq"], f32)
    bv = np.asarray(inputs["bv"], f32)
    bo = np.asarray(inputs["bo"], f32)
    # NOTE: bk is dropped intentionally — it adds q.bk to every logit of a
    # given query (constant across keys), which softmax cancels exactly.

    s = f32(1.0) / np.sqrt(f32(D))
    A_h = ((Wq.T @ Wk) * s).astype(bf16)                 # [d, d']
    Ct_h = np.ascontiguousarray((Wo @ Wv).T).astype(bf16)  # [d, f]
    cb_h = (qe @ (Wk.T @ bq) * s).astype(f32)            # [B, Lk]
    bof_h = (bo + Wo @ bv).reshape(D, 1).astype(f32)

    xT = np.ascontiguousarray(qe.transpose(0, 2, 1)).astype(bf16)  # [B, D, Lk]
    xN = qe.astype(bf16)                                           # [B, Lk, D]
    gT = np.ascontiguousarray(ge.transpose(0, 2, 1)).astype(bf16)  # [B, D, Lq]

    in_maps = []
    for c in range(NCORES):
        bs = slice(c * BPC, (c + 1) * BPC)
        in_maps.append({
            "xT": xT[bs], "xN": xN[bs], "gT": gT[bs], "cb": cb_h[bs],
            "A": A_h, "Ct": Ct_h, "bof": bof_h,
        })
    return in_maps


def _run(inputs, trace=False, **kw):
    nc = build_nc()
    in_maps = _prep_in_maps(inputs)
    res = run_bass_kernel_spmd(nc, in_maps, list(range(NCORES)),
                               trace=trace, **kw)
    outT = np.stack([r["outT"] for r in res.results])   # [8, BPC, D, Lq]
    out = np.ascontiguousarray(outT.transpose(0, 1, 3, 2)).reshape(B, LQ, D)
    return out, res


def kernel(**inputs) -> np.ndarray:
    out, _ = _run(inputs)
    return out


# revision 11
# speedup vs baseline: 1.0680x; 1.0680x over previous
# Cross-attention (single head) kernel for Trainium2, SPMD over 8 NeuronCores.
#
# Problem: nn_MultiHeadedAttention — B=16, Lq=1024, Lk=2048, D=768, fp32.
#   q = guide @ Wq.T + bq ; k = query @ Wk.T + bk ; v = query @ Wv.T + bv
#   out = softmax(q k^T / sqrt(D)) v ;  result = out @ Wo.T + bo
#
# Sharding: data-parallel over batch (2 batches per core), no collectives.
#
# Algebraic folding (host-side, exact):
#   - bk adds a per-query constant to all logits -> softmax-invariant, drop.
#   - A := Wq^T Wk / sqrt(D):  S = guide A query^T + c,  c = query (Wk^T bq)/sqrt(D)
#     (c is per-key, host-precomputed, applied as the exp() activation bias).
#     This eliminates both the q and k projections (one folded projection).
#   - C := Wo Wv:  result = (P_hat query) C^T + (bo + Wo bv)
#     This eliminates the v projection entirely; PV consumes raw query
#     embeds. Normalization by the softmax denominator is deferred to after
#     the C projection (linearity), so the reciprocal never blocks the PE.
#
# Layout strategy (zero on-device transposes): host uploads query embeds in
# both layouts (xT=[D,Lk] for S^T, x_nat=[Lk,D] for PV) and guide as
# gT=[D,Lq]; kernel computes S^T=[ik,iq] tiles so softmax denominators come
# from an all-ones [128,128] stationary matmul that directly produces the
# denominator broadcast across all partitions.
#
# All matmuls are bf16 operands with fp32 PSUM accumulation (measured
# end-to-end rel err vs fp32 reference ~3e-3).

import numpy as np
import ml_dtypes
from contextlib import ExitStack

import concourse.bass as bass
import concourse.tile as tile
from concourse import bacc, mybir
from concourse.bass_utils import run_bass_kernel_spmd

BF16 = mybir.dt.bfloat16
F32 = mybir.dt.float32

B, LQ, LK, D = 16, 1024, 2048, 768
NCORES = 8
BPC = B // NCORES          # batches per core = 2
NT = D // 128              # 6 feature tiles of 128
QC = 512                   # query chunk (free dim of S^T / PSUM bank width)
NQC = LQ // QC             # 2 query chunks
NIK = LK // 128            # 16 key tiles of 128
ACT_F = mybir.ActivationFunctionType


def build_nc():
    nc = bacc.Bacc("TRN2", target_bir_lowering=False, debug=False,
                   num_devices=NCORES)

    xT = nc.dram_tensor("xT", [BPC, D, LK], BF16, kind="ExternalInput").ap()
    xN = nc.dram_tensor("xN", [BPC, LK, D], BF16, kind="ExternalInput").ap()
    gT = nc.dram_tensor("gT", [BPC, D, LQ], BF16, kind="ExternalInput").ap()
    A = nc.dram_tensor("A", [D, D], BF16, kind="ExternalInput").ap()
    Ct = nc.dram_tensor("Ct", [D, D], BF16, kind="ExternalInput").ap()
    cb = nc.dram_tensor("cb", [BPC, LK], F32, kind="ExternalInput").ap()
    bof = nc.dram_tensor("bof", [D, 1], F32, kind="ExternalInput").ap()
    outT = nc.dram_tensor("outT", [BPC, D, LQ], F32, kind="ExternalOutput").ap()

    with tile.TileContext(nc) as tc, ExitStack() as ctx:
        _body(ctx, tc, outT, xT, xN, gT, A, Ct, cb, bof)
    nc.compile()
    return nc


def _body(ctx, tc, outT, xT, xN, gT, A, Ct, cb, bof):
    nc = tc.nc

    wts = ctx.enter_context(tc.tile_pool(name="wts", bufs=1))
    consts = ctx.enter_context(tc.tile_pool(name="consts", bufs=1))
    io = ctx.enter_context(tc.tile_pool(name="io", bufs=2))
    kqv = ctx.enter_context(tc.tile_pool(name="kqv", bufs=1))
    pt_pool = ctx.enter_context(tc.tile_pool(name="pt", bufs=17))
    ot_pool = ctx.enter_context(tc.tile_pool(name="ot", bufs=2))
    res_pool = ctx.enter_context(tc.tile_pool(name="res", bufs=4))
    rb_pool = ctx.enter_context(tc.tile_pool(name="rb", bufs=2))
    psum = ctx.enter_context(tc.tile_pool(name="psum", bufs=1, space="PSUM"))

    # --- weights/constants tiles (DMAs emitted in first-use order below) ---
    A_sb = wts.tile([128, NT, D], BF16, tag="A", name="A_sb")
    Ct_sb = wts.tile([128, NT, D], BF16, tag="Ct", name="Ct_sb")
    bof_sb = consts.tile([128, NT], F32, tag="bof", name="bof_sb")
    ones_sb = consts.tile([128, 128], BF16, tag="ones", name="ones_sb")
    nc.vector.memset(ones_sb, 1.0)

    for b in range(BPC):
        # --- load activations, ordered by first use so the PE can start as
        # soon as A + gT land (qp projection), then xT (S^T), then xN (PV),
        # then Ct (final projection) ---
        xT_sb = io.tile([128, NT, LK], BF16, tag="xT", name="xT_sb")
        xN_sb = io.tile([128, NIK, D], BF16, tag="xN", name="xN_sb")
        gT_sb = io.tile([128, NT, LQ], BF16, tag="gT", name="gT_sb")
        cb_sb = io.tile([128, NIK], F32, tag="cb", name="cb_sb")
        if b == 0:
            for td in range(NT):
                r = slice(td * 128, (td + 1) * 128)
                nc.sync.dma_start(out=A_sb[:, td, :], in_=A[r, :])
        for qc in range(NQC):
            cc = slice(qc * QC, (qc + 1) * QC)
            for td in range(NT):
                r = slice(td * 128, (td + 1) * 128)
                nc.sync.dma_start(out=gT_sb[:, td, cc], in_=gT[b, r, cc])
        for td in range(NT):
            r = slice(td * 128, (td + 1) * 128)
            nc.sync.dma_start(out=xT_sb[:, td, :], in_=xT[b, r, :])
        nc.sync.dma_start(out=cb_sb, in_=cb[b].rearrange("(t p) -> p t", p=128))
        for ik in range(NIK):
            r = slice(ik * 128, (ik + 1) * 128)
            nc.sync.dma_start(out=xN_sb[:, ik, :], in_=xN[b, r, :])
        if b == 0:
            for td in range(NT):
                r = slice(td * 128, (td + 1) * 128)
                nc.sync.dma_start(out=Ct_sb[:, td, :], in_=Ct[r, :])
            nc.sync.dma_start(out=bof_sb,
                             in_=bof.rearrange("(t p) o -> p (t o)", p=128))

        # --- folded q projection: qp^T[d', iq] = sum_d A[d, d'] gT[d, iq] ---
        qp_sb = kqv.tile([128, NT, LQ], BF16, tag="qp", name="qp_sb")
        for qc in range(NQC):
            cc = slice(qc * QC, (qc + 1) * QC)
            for te in range(NT):
                ec = slice(te * 128, (te + 1) * 128)
                ps = psum.tile([128, QC], F32, tag="acc", bufs=3, name="ps_q")
                for td in range(NT):
                    nc.tensor.matmul(ps, A_sb[:, td, ec], gT_sb[:, td, cc],
                                     start=(td == 0), stop=(td == NT - 1))
                nc.vector.tensor_copy(qp_sb[:, te, cc], ps)

        # --- attention, per query chunk of 512 ---
        for qc in range(NQC):
            cc = slice(qc * QC, (qc + 1) * QC)

            # denominator, broadcast over all 128 partitions by the all-ones
            # stationary operand: dn[m, iq] = sum_ik P^T[ik, iq] for every m
            dn = psum.tile([128, QC], F32, tag="dn", bufs=1, name="dn")
            ps_o = [psum.tile([128, QC], F32, tag="acc", bufs=3,
                              name=f"ps_o{t}") for t in range(3)]
            pts = []
            for ik in range(NIK):
                kc = slice(ik * 128, (ik + 1) * 128)
                # S^T[ik, iq] = sum_d' xT[d', ik] * qp^T[d', iq]
                ps_s = psum.tile([128, QC], F32, tag="s", bufs=3, name="ps_s")
                for te in range(NT):
                    nc.tensor.matmul(ps_s, xT_sb[:, te, kc], qp_sb[:, te, cc],
                                     start=(te == 0), stop=(te == NT - 1))
                # P^T = exp(S^T + c[ik])   (c: folded bq term, per-key)
                pt = pt_pool.tile([128, QC], BF16, tag="pt", name="pt")
                nc.scalar.activation(pt, ps_s, ACT_F.Exp,
                                     bias=cb_sb[:, ik:ik + 1])
                pts.append(pt)
                nc.tensor.matmul(dn, ones_sb, pt,
                                 start=(ik == 0), stop=(ik == NIK - 1))
                # PV wave 1: out^T[d, iq] += xN[ik, d] * P^T[ik, iq], d-tiles 0-2
                for t in range(3):
                    dc = slice(t * 128, (t + 1) * 128)
                    nc.tensor.matmul(ps_o[t], xN_sb[:, ik, dc], pt,
                                     start=(ik == 0), stop=(ik == NIK - 1))

            # wave-1 copies first: they release the acc PSUM slots wave 2
            # needs; the reciprocal (slow on DVE) runs after, overlapped
            # with wave-2 matmuls — it is only needed at the final muls.
            oT = ot_pool.tile([128, NT, QC], BF16, tag="oT", name="oT")
            for t in range(3):
                nc.vector.tensor_copy(oT[:, t, :], ps_o[t])

            # PV wave 2: d-tiles 3-5
            ps_o2 = [psum.tile([128, QC], F32, tag="acc", bufs=3,
                               name=f"ps_o2{t}") for t in range(3)]
            for ik in range(NIK):
                for t in range(3):
                    dc = slice((t + 3) * 128, (t + 4) * 128)
                    nc.tensor.matmul(ps_o2[t], xN_sb[:, ik, dc], pts[ik],
                                     start=(ik == 0), stop=(ik == NIK - 1))
            for t in range(3):
                nc.vector.tensor_copy(oT[:, t + 3, :], ps_o2[t])
            # fast-approx reciprocal (~18 bits, ~5x faster than exact):
            # denominators are ~2e3 (positive, normal), far from the
            # undefined edge cases, and 4e-6 rel err is negligible against
            # the bf16 matmul error. Keeping this op short keeps the DVE
            # free for the CASTs that release PSUM slots the PE waits on.
            rb = rb_pool.tile([128, QC], F32, tag="rb", name="rb")
            nc.vector.reciprocal_approx_fast(out=rb, in_=dn)

            # result^T[f, iq] = (sum_d Ct[d, f] oT[d, iq]) / denom + bo'
            for tf in range(NT):
                fc = slice(tf * 128, (tf + 1) * 128)
                ps_r = psum.tile([128, QC], F32, tag="acc", bufs=3,
                                 name="ps_r")
                for te in range(NT):
                    nc.tensor.matmul(ps_r, Ct_sb[:, te, fc], oT[:, te, :],
                                     start=(te == 0), stop=(te == NT - 1))
                res = res_pool.tile([128, QC], F32, tag="res", name="res")
                nc.vector.tensor_mul(res, ps_r, rb)
                nc.scalar.activation(res, res, ACT_F.Identity,
                                     bias=bof_sb[:, tf:tf + 1])
                nc.sync.dma_start(out=outT[b, fc, cc], in_=res)


def _prep_in_maps(inputs):
    f32 = np.float32
    bf16 = ml_dtypes.bfloat16
    qe = np.asarray(inputs["query_embeds"], f32)    # [B, Lk, D]
    ge = np.asarray(inputs["guide_embeds"], f32)    # [B, Lq, D]
    Wq = np.asarray(inputs["Wq"], f32)
    Wk = np.asarray(inputs["Wk"], f32)
    Wv = np.asarray(inputs["Wv"], f32)
    Wo = np.asarray(inputs["Wo"], f32)
    bq = np.asarray(inputs["bq"], f32)
    bv = np.asarray(inputs["bv"], f32)
    bo = np.asarray(inputs["bo"], f32)
    # NOTE: bk is dropped intentionally — it adds q.bk to every logit of a
    # given query (constant across keys), which softmax cancels exactly.

    s = f32(1.0) / np.sqrt(f32(D))
    A_h = ((Wq.T @ Wk) * s).astype(bf16)                 # [d, d']
    Ct_h = np.ascontiguousarray((Wo @ Wv).T).astype(bf16)  # [d, f]
    cb_h = (qe @ (Wk.T @ bq) * s).astype(f32)            # [B, Lk]
    bof_h = (bo + Wo @ bv).reshape(D, 1).astype(f32)

    xT = np.ascontiguousarray(qe.transpose(0, 2, 1)).astype(bf16)  # [B, D, Lk]
    xN = qe.astype(bf16)                                           # [B, Lk, D]
    gT = np.ascontiguousarray(ge.transpose(0, 2, 1)).astype(bf16)  # [B, D, Lq]

    in_maps = []
    for c in range(NCORES):
        bs = slice(c * BPC, (c + 1) * BPC)
        in_maps.append({
            "xT": xT[bs], "xN": xN[bs], "gT": gT[bs], "cb": cb_h[bs],
            "A": A_h, "Ct": Ct_h, "bof": bof_h,
        })
    return in_maps


def _run(inputs, trace=False, **kw):
    nc = build_nc()
    in_maps = _prep_in_maps(inputs)
    res = run_bass_kernel_spmd(nc, in_maps, list(range(NCORES)),
                               trace=trace, **kw)
    outT = np.stack([r["outT"] for r in res.results])   # [8, BPC, D, Lq]
    out = np.ascontiguousarray(outT.transpose(0, 1, 3, 2)).reshape(B, LQ, D)
    return out, res


def kernel(**inputs) -> np.ndarray:
    out, _ = _run(inputs)
    return out
